# revision 41
# baseline (speedup 1.0000x reference)
"""Gated dual-score (semantic+geometric/RoPE) causal attention layer on 8 TRN2 cores.

Sharding: data-parallel over batch (2) x tensor-parallel over heads (16 -> 4/core).
Core i: batch b = i // 4, heads hg = i % 4 -> heads [4*hg, 4*hg+4).
Each core computes a partial y (its heads' contribution, its batch); the host
sums the 4 partials per batch (the "all-reduce" of the row-sharded out-proj).

On-device layout: all projections consume xT (d on partitions, t free) and
produce qT/kT in (d, t) layout; scores are [s, t] so sem+geo fuse into one
128-contraction matmul, the gate folds into a pre-scale of q columns, and
P@V consumes exp tiles directly with V in natural (t, dv) layout.

v2 restructure vs baseline:
  - chunk-outer / head-inner pipeline: per 512-token chunk j do gate(j),
    V(j), QK(all heads, j), then attention, then out-proj. Chunk-0 work
    overlaps the input DMA ramp instead of idling.
  - attention runs two heads interleaved (pairs), so one head's exp
    latency hides under the other head's score/PV matmuls.
  - out-proj of chunk j-1 is woven matmul-by-matmul into chunk j's
    attention streams as PE filler for the remaining exp bubbles; it no
    longer tails the kernel.
  - softmax denominator: exp tiles are accumulated on the Vector engine
    (bf16, 4x mode) into one [128,512] tile per (h,j); a single ones-MATRIX
    matmul then produces the denominator already broadcast across all 128
    partitions (PSUM). Replaces per-s-tile ones-vector matmuls + the
    broadcast matmul + copies: ~70k fewer PE cycles.
  - y PSUM->SBUF copies run on the otherwise-idle GPSIMD engine.
  - xt is double-buffered per chunk (2x16KB/partition instead of 64KB).
Softmax skips max-subtraction (scores are O(5) by construction).
Compute dtype bf16 (fp32 matmul costs 4x cycles on TRN2), fp32 accumulation.
"""

import sys
from contextlib import ExitStack

import numpy as np

sys.path.insert(0, "/opt/trn_rl_repo")

import ml_dtypes  # noqa: E402

import concourse.bass as bass  # noqa: E402
from concourse import bacc  # noqa: E402
import concourse.mybir as mybir  # noqa: E402
import concourse.tile as tile  # noqa: E402
from concourse.bass_utils import run_bass_kernel_spmd  # noqa: E402

B, T, D, H = 2, 2048, 2048, 16
SEM_HD = GEO_HD = 64
V_HD = 128
HL = 4  # heads per core
CL = HL * V_HD  # local v-dim (512)
ROPE_BASE = 10000.0

KT = D // 128  # 16 k-tiles over the contraction dim
TT = T // 128  # 16 token tiles of 128
TC = T // 512  # 4 token chunks of 512
BF = mybir.dt.bfloat16
F32 = mybir.dt.float32
NPBF = ml_dtypes.bfloat16

_CACHED_NC = None


def _build_nc():
    nc = bacc.Bacc()

    xt_d = nc.declare_dram_parameter("xt", [128, KT, TC, 512], BF, isOutput=False)
    wq_d = nc.declare_dram_parameter("wq", [HL, 128, KT, 128], BF, isOutput=False)
    wk_d = nc.declare_dram_parameter("wk", [HL, 128, KT, 128], BF, isOutput=False)
    wv_d = nc.declare_dram_parameter("wv", [128, KT, CL], BF, isOutput=False)
    wo_d = nc.declare_dram_parameter("wo", [HL, 128, D], BF, isOutput=False)
    wg_d = nc.declare_dram_parameter("wg", [128, KT, 2 * HL], BF, isOutput=False)
    glog_d = nc.declare_dram_parameter("glog", [2 * HL, 1], F32, isOutput=False)
    crep_d = nc.declare_dram_parameter("crep", [128, T], BF, isOutput=False)
    srep_d = nc.declare_dram_parameter("srep", [128, T], BF, isOutput=False)
    maskd_d = nc.declare_dram_parameter("maskd", [128, 128], BF, isOutput=False)
    selc_d = nc.declare_dram_parameter("selc", [2 * HL, HL * 128], BF, isOutput=False)
    gsv_d = nc.declare_dram_parameter("gsv", [2 * HL, 1], F32, isOutput=False)
    gbv_d = nc.declare_dram_parameter("gbv", [2 * HL, 1], F32, isOutput=False)
    y_d = nc.declare_dram_parameter("y", [T, D], BF, isOutput=True)

    with tile.TileContext(nc) as tc, ExitStack() as ctx:
        singles = ctx.enter_context(tc.tile_pool(name="singles", bufs=1))
        xpool = ctx.enter_context(tc.tile_pool(name="xpool", bufs=2))
        vpool = ctx.enter_context(tc.tile_pool(name="vpool", bufs=1))
        wpool = ctx.enter_context(tc.tile_pool(name="wpool", bufs=1))
        qs_pool = ctx.enter_context(tc.tile_pool(name="qs", bufs=4))
        kst_pool = ctx.enter_context(tc.tile_pool(name="kst", bufs=1))
        ot_pool = ctx.enter_context(tc.tile_pool(name="ot", bufs=2))
        p_pool = ctx.enter_context(tc.tile_pool(name="pp", bufs=8))
        acc_pool = ctx.enter_context(tc.tile_pool(name="accp", bufs=4))
        sc_pool = ctx.enter_context(tc.tile_pool(name="scratch", bufs=2))
        y_pool = ctx.enter_context(tc.tile_pool(name="ysb", bufs=4))

        ps_a = ctx.enter_context(tc.tile_pool(name="ps_a", bufs=2, space="PSUM"))
        ps_qk = ctx.enter_context(tc.tile_pool(name="ps_qk", bufs=2, space="PSUM"))
        ps_s = ctx.enter_context(tc.tile_pool(name="ps_s", bufs=2, space="PSUM"))
        ps_o = ctx.enter_context(tc.tile_pool(name="ps_o", bufs=2, space="PSUM"))

        # ---- DMA priority order ----
        wg = singles.tile([128, KT, 2 * HL], BF)
        nc.sync.dma_start(out=wg, in_=wg_d[:])

        xts = {}

        def load_xt(j, split=1):
            t = xpool.tile([128, KT, 512], BF, tag="xt", name=f"xt{j}")
            kstep = KT // split
            for k0 in range(0, KT, kstep):
                nc.sync.dma_start(
                    out=t[:, k0 : k0 + kstep, :],
                    in_=xt_d[:, k0 : k0 + kstep, j, :],
                )
            xts[j] = t

        load_xt(0, split=4)

        wq_sb = [wpool.tile([128, KT, 128], BF, name=f"wq{h}") for h in range(HL)]
        wk_sb = [wpool.tile([128, KT, 128], BF, name=f"wk{h}") for h in range(HL)]
        nc.sync.dma_start(out=wq_sb[0], in_=wq_d[0])
        nc.sync.dma_start(out=wk_sb[0], in_=wk_d[0])

        crep = singles.tile([128, T], BF)
        srep = singles.tile([128, T], BF)
        maskd = singles.tile([128, 128], BF)
        nc.sync.dma_start(out=crep, in_=crep_d[:])
        nc.sync.dma_start(out=srep, in_=srep_d[:])
        nc.sync.dma_start(out=maskd, in_=maskd_d[:])
        glog = singles.tile([2 * HL, 1], F32)
        nc.sync.dma_start(out=glog, in_=glog_d[:])
        selc = singles.tile([2 * HL, HL * 128], BF)
        nc.sync.dma_start(out=selc, in_=selc_d[:])
        gsv = singles.tile([2 * HL, 1], F32)
        gbv = singles.tile([2 * HL, 1], F32)
        nc.sync.dma_start(out=gsv, in_=gsv_d[:])
        nc.sync.dma_start(out=gbv, in_=gbv_d[:])

        nc.sync.dma_start(out=wq_sb[1], in_=wq_d[1])
        nc.sync.dma_start(out=wk_sb[1], in_=wk_d[1])

        wv = wpool.tile([128, KT, CL], BF, name="wv")
        nc.sync.dma_start(out=wv, in_=wv_d[:])

        load_xt(1)
        for h in range(2, HL):
            nc.sync.dma_start(out=wq_sb[h], in_=wq_d[h])
            nc.sync.dma_start(out=wk_sb[h], in_=wk_d[h])

        wo_sb = [wpool.tile([128, D], BF, name=f"wo{h}") for h in range(HL)]
        for h in range(HL):
            nc.sync.dma_start(out=wo_sb[h], in_=wo_d[h])

        # ones matrix: lhsT for the denominator matmul; M=128 means the
        # output has the column-sum broadcast down all 128 partitions.
        ones_mat = singles.tile([128, 128], BF)
        nc.vector.memset(ones_mat, 1.0)

        # persistent per-head k (scores for chunk j read s-tiles 0..4j+3)
        kstk = [kst_pool.tile([128, T], BF, name=f"kstk{h}") for h in range(HL)]
        v_sb = vpool.tile([128, TT, CL], BF)
        gcomb = singles.tile([2 * HL, T], BF)

        def emit_outproj(j):
            """Out-projection of chunk j as a list of thunks: 16 PSUM-
            accumulating matmul groups of 4, each followed by copy+DMA."""
            outT = outTs[j]
            steps = []
            for r in range(4):
                i = 4 * j + r
                for ec in range(D // 512):
                    def step(r=r, i=i, ec=ec):
                        py = ps_a.tile([128, 512], F32, tag="big", name="py")
                        for h in range(HL):
                            nc.tensor.matmul(
                                py,
                                outT[h][:, 128 * r : 128 * (r + 1)],
                                wo_sb[h][:, 512 * ec : 512 * (ec + 1)],
                                start=(h == 0),
                                stop=(h == HL - 1),
                            )
                        ysb = y_pool.tile([128, 512], BF, tag="ysb")
                        # alternate copy engine so neither scalar nor DVE
                        # serializes the y drain
                        if (r + ec) % 2 == 0:
                            nc.vector.tensor_copy(ysb, py)
                        else:
                            nc.scalar.copy(ysb, py)
                        nc.sync.dma_start(
                            out=y_d[
                                128 * i : 128 * (i + 1), 512 * ec : 512 * (ec + 1)
                            ],
                            in_=ysb,
                        )
                    steps.append(step)
            return steps

        outTs = {}

        # ---- gate projection chunk j: gcomb rows 0:4 = g/8, 4:8 = (1-g)/8
        # sigmoid computed as 1/(1+exp(-z)) with the exp on the scalar
        # engine (avoids sigmoid<->exp activation-table reloads; glog
        # arrives negated from the host) and the reciprocal on DVE.
        def make_gate(j):
            def run():
                tsl = slice(512 * j, 512 * (j + 1))
                xt = xts[j]
                pg = ps_s.tile([2 * HL, 512], F32, tag="ps", name="pg")
                for k in range(KT):
                    nc.tensor.matmul(
                        pg, wg[:, k, :], xt[:, k, :],
                        start=(k == 0), stop=(k == KT - 1),
                    )
                gsig = sc_pool.tile([2 * HL, 512], F32, tag="gsig", bufs=1)
                nc.scalar.activation(
                    gsig, pg, mybir.ActivationFunctionType.Exp, scale=-1.0, bias=glog
                )
                gw = sc_pool.tile([2 * HL, 512], F32, tag="gw", bufs=1)
                nc.vector.tensor_scalar_add(gw, gsig, 1.0)
                grec = sc_pool.tile([2 * HL, 512], F32, tag="grec", bufs=1)
                nc.vector.reciprocal_approx_fast(out=grec, in_=gw)
                nc.scalar.activation(
                    gcomb[:, tsl],
                    grec,
                    mybir.ActivationFunctionType.Identity,
                    scale=gsv,
                    bias=gbv,
                )
            return run

        # ---- V projection chunk j: 4 t-tiles, natural (t, dv) layout
        def make_v_steps(j):
            steps = []
            for r in range(4):
                def st(r=r):
                    xt = xts[j]
                    i = 4 * j + r
                    pv = ps_a.tile([128, CL], F32, tag="big", name="pv")
                    for k in range(KT):
                        nc.tensor.matmul(
                            pv,
                            xt[:, k, 128 * r : 128 * (r + 1)],
                            wv[:, k, :],
                            start=(k == 0),
                            stop=(k == KT - 1),
                        )
                    nc.scalar.copy(v_sb[:, i, :], pv)
                steps.append(st)
            return steps

        gv = []  # pending gate/V thunks for the next chunk (fill first)
        op = []  # pending out-proj steps of the previous chunk

        for j in range(TC):
            tsl = slice(512 * j, 512 * (j + 1))
            xt = xts[j]

            # ---- QK projection + RoPE/gate for one head
            qstks = {}

            def emit_qk(h):
                pq = ps_qk.tile([128, 512], F32, tag="qk", name="pq")
                pk = ps_qk.tile([128, 512], F32, tag="qk", name="pk")
                for k in range(KT):
                    nc.tensor.matmul(
                        pq, wq_sb[h][:, k, :], xt[:, k, :],
                        start=(k == 0), stop=(k == KT - 1),
                    )
                for k in range(KT):
                    nc.tensor.matmul(
                        pk, wk_sb[h][:, k, :], xt[:, k, :],
                        start=(k == 0), stop=(k == KT - 1),
                    )
                # gate broadcast: rows 0:64 <- g/8, rows 64:128 <- (1-g)/8
                gbb = ps_s.tile([128, 512], F32, tag="ps", name="gbb")
                nc.tensor.matmul(
                    gbb,
                    selc[:, 128 * h : 128 * (h + 1)],
                    gcomb[:, tsl],
                    start=True,
                    stop=True,
                )
                gbs = sc_pool.tile([128, 512], BF, tag="gbs", bufs=2)
                nc.scalar.copy(gbs, gbb)

                # stash pq/pk to bf16 SBUF immediately (scalar), freeing the
                # PSUM banks fast AND letting every RoPE mul run in the DVE
                # all-bf16-SBUF fast mode (4x) instead of reading PSUM fp32.
                pqb = sc_pool.tile([128, 512], BF, tag="sem", bufs=2)
                nc.scalar.copy(pqb, pq)
                nc.scalar.copy(kstk[h][0:64, tsl], pk[0:64, :])
                pkg = sc_pool.tile([128, 512], BF, tag="kgeo", bufs=2)
                nc.scalar.copy(pkg[64:128, :], pk[64:128, :])
                m1 = sc_pool.tile([128, 512], BF, tag="m1", bufs=4)
                m2 = sc_pool.tile([128, 512], BF, tag="m2", bufs=4)
                m2b = sc_pool.tile([128, 512], BF, tag="m2", bufs=4)
                nc.vector.tensor_mul(m1[64:128, :], pqb[64:128, :], crep[64:128, tsl])
                nc.vector.tensor_mul(m2[64:128, :], pqb[64:128, :], srep[64:128, tsl])
                km1 = sc_pool.tile([128, 512], BF, tag="m1", bufs=4)
                km2 = sc_pool.tile([128, 512], BF, tag="m2", bufs=4)
                km2b = sc_pool.tile([128, 512], BF, tag="m2", bufs=4)
                nc.vector.tensor_mul(km1[64:128, :], pkg[64:128, :], crep[64:128, tsl])
                nc.vector.tensor_mul(km2[64:128, :], pkg[64:128, :], srep[64:128, tsl])
                # rotate halves (half-split RoPE): x1*cos -/+ swapped x2*sin
                nc.vector.tensor_copy(m2b[64:96, :], m2[96:128, :])
                nc.vector.tensor_copy(m2b[96:128, :], m2[64:96, :])
                nc.vector.tensor_sub(m1[64:96, :], m1[64:96, :], m2b[64:96, :])
                nc.vector.tensor_add(m1[96:128, :], m1[96:128, :], m2b[96:128, :])
                nc.vector.tensor_copy(km2b[64:96, :], km2[96:128, :])
                nc.vector.tensor_copy(km2b[96:128, :], km2[64:96, :])
                nc.vector.tensor_sub(
                    kstk[h][64:96, tsl], km1[64:96, :], km2b[64:96, :]
                )
                nc.vector.tensor_add(
                    kstk[h][96:128, tsl], km1[96:128, :], km2b[96:128, :]
                )
                # gate scaling of q (all-bf16 SBUF -> DVE fast mode)
                qstk = qs_pool.tile([128, 512], BF, tag="qstk")
                nc.vector.tensor_mul(qstk[0:64, :], pqb[0:64, :], gbs[0:64, :])
                nc.vector.tensor_mul(qstk[64:128, :], m1[64:128, :], gbs[64:128, :])
                qstks[h] = qstk

            # chunk 0 runs during the DMA ramp: emit in weight-arrival order
            # (wq0/wk0 land before wv) so the PE never waits on a transfer
            # that is queued behind another one. Later chunks' gate/V arrive
            # via the gv filler queue during the previous attention.
            if j == 0:
                make_gate(0)()
                emit_qk(0)
                emit_qk(1)
                for st in make_v_steps(0):
                    st()
                emit_qk(2)
                emit_qk(3)
            else:
                for t in gv:  # leftover gate/V of this chunk
                    t()
                gv = []
                for h in range(HL):
                    emit_qk(h)

            # prefetch xt two chunks ahead (now that all xt(j) readers exist,
            # the buffer recycle is safe)
            if j + 2 < TC:
                load_xt(j + 2)
            # next chunk's gate+V fill this chunk's attention bubbles first
            if j + 1 < TC:
                gv = [make_gate(j + 1)] + make_v_steps(j + 1)

            # ---- attention: heads interleaved in pairs; out-proj of the
            # previous chunk woven in as PE filler for exp-latency bubbles
            outT = [
                ot_pool.tile([128, 512], BF, tag=f"ot{h}", name=f"ot{h}_{j}")
                for h in range(HL)
            ]
            outTs[j] = outT
            n_s = 4 * (j + 1)
            slot = 0  # filler slot counter across both pairs
            for pair in range(2):
                hh = (2 * pair, 2 * pair + 1)
                po = {
                    h: ps_o.tile([128, 512], F32, tag="po", name=f"po{h}")
                    for h in hh
                }
                # two denominator accumulators per head (even/odd s) halve the
                # serial DVE add chain; the ones-matmul sums both via PSUM.
                acc = {
                    (h, p): acc_pool.tile(
                        [128, 512], BF, tag="acc", name=f"acc{h}_{p}"
                    )
                    for h in hh
                    for p in range(2)
                }
                for s in range(n_s):
                    dj = s - 4 * j  # >=0 on diagonal tiles
                    c0 = 128 * dj if dj >= 0 else 0
                    ssl = slice(128 * s, 128 * (s + 1))
                    pts = {}
                    for h in hh:
                        pss = ps_s.tile([128, 512], F32, tag="ps", name="ps")
                        nc.tensor.matmul(
                            pss[:, c0:512],
                            kstk[h][:, ssl],
                            qstks[h][:, c0:512],
                            start=True,
                            stop=True,
                        )
                        pt = p_pool.tile([128, 512], BF, tag="pt", name="pt")
                        nc.scalar.activation(
                            pt[:, c0:512],
                            pss[:, c0:512],
                            mybir.ActivationFunctionType.Exp,
                        )
                        if dj >= 0:
                            nc.vector.tensor_mul(
                                pt[:, c0 : c0 + 128], pt[:, c0 : c0 + 128], maskd
                            )
                        a = acc[(h, s % 2)]
                        if s < 2:
                            if c0 > 0:
                                nc.vector.memset(a[:, 0:c0], 0.0)
                            nc.vector.tensor_copy(a[:, c0:512], pt[:, c0:512])
                        else:
                            nc.vector.tensor_add(
                                a[:, c0:512], a[:, c0:512], pt[:, c0:512]
                            )
                        pts[h] = pt
                    # PE filler between the score pair and the PV pair: the
                    # next chunk's gate/V then the previous chunk's out-proj
                    # run while exp latency drains, keeping the PE busy. On
                    # the last chunk there is no next-chunk work, so pace the
                    # 16 out-proj steps across all 32 slots (both pairs)
                    # instead of exhausting them in pair 0.
                    if gv:
                        gv.pop(0)()
                    elif op and (j < TC - 1 or slot % 2 == 0):
                        op.pop(0)()
                    slot += 1
                    for h in hh:
                        nc.tensor.matmul(
                            po[h][:, c0:512],
                            v_sb[:, s, 128 * h : 128 * (h + 1)],
                            pts[h][:, c0:512],
                            start=(s == 0),
                            stop=(s == n_s - 1),
                        )
                for h in hh:
                    # stash po to bf16 SBUF right away (frees the PSUM bank for
                    # the next pair without waiting on the normalize chain)
                    nc.scalar.copy(outT[h], po[h])
                    # denominator: ones-matrix matmuls -> broadcast column sums
                    pdb = ps_qk.tile([128, 512], F32, tag="qk", name="pdb")
                    nc.tensor.matmul(
                        pdb, ones_mat, acc[(h, 0)], start=True, stop=False
                    )
                    nc.tensor.matmul(
                        pdb, ones_mat, acc[(h, 1)], start=False, stop=True
                    )
                    rbs = sc_pool.tile([128, 512], F32, tag="rbs")
                    nc.vector.reciprocal_approx_fast(out=rbs, in_=pdb)
                    nc.vector.tensor_mul(outT[h], outT[h], rbs)

            for step in op:  # drain any leftover previous out-proj
                step()
            op = emit_outproj(j)

        for step in op:  # final chunk's out-projection
            step()

    nc.finalize()
    return nc


def _host_prep(x, w_q_sem, w_k_sem, w_q_geo, w_k_geo, w_v, w_out, gate_logit, gate_w):
    """Build the 8 per-core input maps (all numpy, bf16 where matmul-bound)."""
    half = GEO_HD // 2  # 32
    inv_freq = 1.0 / (ROPE_BASE ** (np.arange(half, dtype=np.float64) / half))
    pos = np.arange(T, dtype=np.float64)
    ang = pos[None, :] * inv_freq[:, None]  # (32, T)
    crep = np.zeros((128, T), dtype=NPBF)
    srep = np.zeros((128, T), dtype=NPBF)
    crep[64:96] = np.cos(ang)
    crep[96:128] = np.cos(ang)
    srep[64:96] = np.sin(ang)
    srep[96:128] = np.sin(ang)

    p_i = np.arange(128)
    maskd = np.where(p_i[:, None] <= p_i[None, :], 1.0, 0.0).astype(NPBF)

    # per-head stacked [sem64 | geo64] projection weights, (128, KT, 128) layout
    def stack_heads(wsem, wgeo):
        out = []
        for h in range(H):
            blk = np.concatenate(
                [wsem[:, 64 * h : 64 * (h + 1)], wgeo[:, 64 * h : 64 * (h + 1)]],
                axis=1,
            )  # (D, 128)
            out.append(
                np.ascontiguousarray(
                    blk.reshape(KT, 128, 128).transpose(1, 0, 2)
                ).astype(NPBF)
            )
        return out  # H x (128, KT, 128)

    wq_all = stack_heads(w_q_sem, w_q_geo)
    wk_all = stack_heads(w_k_sem, w_k_geo)

    in_maps = []
    for core in range(8):
        b, hg = core // 4, core % 4
        heads = range(4 * hg, 4 * hg + 4)
        # xt: [p, k, chunk, 512] so one chunk is a single 3D-DMA
        xt = np.ascontiguousarray(
            x[b].T.astype(NPBF).reshape(KT, 128, TC, 512).transpose(1, 0, 2, 3)
        )
        wq = np.stack([wq_all[h] for h in heads])
        wk = np.stack([wk_all[h] for h in heads])
        wv = np.ascontiguousarray(
            w_v[:, CL * hg : CL * (hg + 1)]
            .reshape(KT, 128, CL)
            .transpose(1, 0, 2)
            .astype(NPBF)
        )
        wo = w_out[CL * hg : CL * (hg + 1), :].reshape(HL, 128, D).astype(NPBF)
        gwl = gate_w[:, 4 * hg : 4 * hg + 4]  # (D, 4)
        gw2 = np.concatenate([gwl, gwl], axis=1)  # (D, 8) duplicated
        wg = np.ascontiguousarray(
            gw2.reshape(KT, 128, 2 * HL).transpose(1, 0, 2)
        ).astype(NPBF)
        selc = np.zeros((2 * HL, HL * 128), dtype=NPBF)
        for h in range(HL):
            selc[h, 128 * h : 128 * h + 64] = 1.0
            selc[HL + h, 128 * h + 64 : 128 * h + 128] = 1.0
        gsv = np.array([0.125] * HL + [-0.125] * HL, dtype=np.float32).reshape(
            2 * HL, 1
        )
        gbv = np.array([0.0] * HL + [0.125] * HL, dtype=np.float32).reshape(2 * HL, 1)
        # negated: device computes sigmoid(z) as 1/(1+exp(-dyn + (-glog)))
        gll = -gate_logit[4 * hg : 4 * hg + 4]
        glog = np.ascontiguousarray(
            np.concatenate([gll, gll]).reshape(2 * HL, 1)
        ).astype(np.float32)
        in_maps.append(
            {
                "xt": xt,
                "wq": wq,
                "wk": wk,
                "wv": np.ascontiguousarray(wv),
                "wo": np.ascontiguousarray(wo),
                "wg": wg,
                "glog": glog,
                "crep": crep,
                "srep": srep,
                "maskd": maskd,
                "selc": selc,
                "gsv": gsv,
                "gbv": gbv,
            }
        )
    return in_maps


def _run(inputs, trace=False):
    global _CACHED_NC
    if _CACHED_NC is None:
        _CACHED_NC = _build_nc()
    in_maps = _host_prep(**{k: np.asarray(v) for k, v in inputs.items()})
    res = run_bass_kernel_spmd(
        _CACHED_NC, in_maps, core_ids=list(range(8)), trace=trace
    )
    y = np.zeros((B, T, D), dtype=np.float32)
    for core in range(8):
        y[core // 4] += res.results[core]["y"].astype(np.float32)
    return y, res


def kernel(**inputs) -> np.ndarray:
    y, _ = _run(inputs, trace=False)
    return y
